# revision 6
# baseline (speedup 1.0000x reference)
"""Distributed RoPE multi-head attention on one TRN2 chip (8 NeuronCores).

kernel(**inputs) -> np.ndarray [2, 4096, 1024]; bf16 TensorE, f32 PSUM.
"""


import math

import numpy as np

import concourse.bass as bass
import concourse.tile as tile
from concourse import bacc, mybir

F32 = mybir.dt.float32
BF16 = mybir.dt.bfloat16
I16 = mybir.dt.int16
EXP = mybir.ActivationFunctionType.Exp

THETA = 500.0
SCALE = 1.0
B, L, DM = 2, 4096, 1024
H, DH = 16, 64
HALF = DH // 2  # 32
N_CORES = 8
PCHUNK = (B * L) // N_CORES

NJ = L // 128          # 32 key tiles per batch
NIC = L // 512         # 8 query chunks of 512 per batch
XC = 512               # xT s-chunk size for projection
NXC = L // XC

# Schraudolph fast-exp on DVE: bf16 bits of exp(0.125*s) as
# round(s*0.125*128/ln2 + C2) written to an int16 view of the p tile.
# Offloads the top DVE_COLS of each [128,1024] exp tile from ScalarE.
DVE_COLS = 256
SCH_C1 = 0.125 * 128.0 / math.log(2.0)
SCH_C2 = 16243.7


def build(dbg_mode=False):
    nc = bacc.Bacc("TRN2", target_bir_lowering=False, debug=False,
                   num_devices=N_CORES)

    xT = nc.dram_tensor("xT", [DM, B * L], F32, kind="ExternalInput").ap()
    w_qk = nc.dram_tensor("w_qk", [DM, 256], F32, kind="ExternalInput").ap()
    b_qk = nc.dram_tensor("b_qk", [256], F32, kind="ExternalInput").ap()
    w_v = nc.dram_tensor("w_v", [DM, 128], F32, kind="ExternalInput").ap()
    b_v = nc.dram_tensor("b_v", [128], F32, kind="ExternalInput").ap()
    w_out = nc.dram_tensor("w_out", [DM, DM], F32, kind="ExternalInput").ap()
    b_out = nc.dram_tensor("b_out", [DM], F32, kind="ExternalInput").ap()
    cos_t = nc.dram_tensor("cos_t", [128, L], F32, kind="ExternalInput").ap()
    sin_t = nc.dram_tensor("sin_t", [128, L], F32, kind="ExternalInput").ap()
    out = nc.dram_tensor("out", [PCHUNK, DM], F32, kind="ExternalOutput").ap()
    dbg = None
    if dbg_mode:
        dbg = {
            "qt0": nc.dram_tensor("dbg_qt0", [128, L], BF16, kind="ExternalOutput").ap(),
            "kt0": nc.dram_tensor("dbg_kt0", [128, L], BF16, kind="ExternalOutput").ap(),
            "va0": nc.dram_tensor("dbg_va0", [128, NJ * 130], BF16, kind="ExternalOutput").ap(),
            "p00": nc.dram_tensor("dbg_p00", [128, 1024], F32, kind="ExternalOutput").ap(),
            "o00": nc.dram_tensor("dbg_o00", [65, 512], F32, kind="ExternalOutput").ap(),
            "a2ain": nc.dram_tensor("dbg_a2ain", [N_CORES * 128, PCHUNK], F32, kind="ExternalOutput").ap(),
            "a2aout": nc.dram_tensor("dbg_a2aout", [N_CORES * 128, PCHUNK], F32, kind="ExternalOutput").ap(),
        }

    with tile.TileContext(nc) as tc:
        _body(nc, tc, xT, w_qk, b_qk, w_v, b_v, w_out, b_out, cos_t, sin_t,
              out, dbg)
    nc.finalize()
    return nc


def _body(nc, tc, xT, w_qk, b_qk, w_v, b_v, w_out, b_out, cos_t, sin_t, out, dbg=None):
    mm = nc.tensor.matmul

    with tc.tile_pool(name="const", bufs=1) as const, \
         tc.tile_pool(name="qkv", bufs=1) as qkv_pool, \
         tc.tile_pool(name="dram", bufs=1, space="DRAM") as dram:

        a2a_in = [dram.tile([N_CORES, 128, 512], BF16, name=f"a2ai{b}")
                  for b in range(B)]
        a2a_out = [dram.tile([N_CORES, 128, 512], BF16, name=f"a2ao{b}")
                   for b in range(B)]

        wqk = const.tile([128, 16 * 128], BF16)
        wqk_r = wqk.rearrange("p (ct dmt c) -> p ct dmt c", ct=2, dmt=8)
        wqk_d = w_qk.rearrange("(dmt p) (ct c) -> p ct dmt c", p=128, ct=2)
        nc.gpsimd.dma_start(wqk_r[:], wqk_d[:])

        wv = const.tile([128, 8 * 128], BF16)
        wv_r = wv.rearrange("p (dmt c) -> p dmt c", dmt=8)
        nc.gpsimd.dma_start(wv_r[:],
                            w_v.rearrange("(dmt p) c -> p dmt c", p=128))

        bqk = const.tile([128, 2], F32)
        nc.sync.dma_start(bqk[:], b_qk.rearrange("(ct p) -> p ct", p=128))
        bv = const.tile([1, 128], BF16)
        nc.gpsimd.dma_start(bv[:], b_v.rearrange("(o c) -> o c", o=1))
        bout = const.tile([1, DM], BF16)
        nc.gpsimd.dma_start(bout[:], b_out.rearrange("(o c) -> o c", o=1))

        wout = const.tile([128, 8 * DM], BF16)
        wout_r = wout.rearrange("p (ct n) -> p ct n", ct=8)

        ones_row = const.tile([1, 128], BF16)
        nc.vector.memset(ones_row[:], 1.0)

        # Warm the ScalarE exp table (~2.7us ACT_TABLE_LOAD) off the
        # critical path, before the first real softmax tile.
        warm = const.tile([1, 8], F32)
        nc.vector.memset(warm[:], 0.0)
        warm2 = const.tile([1, 8], F32)
        nc.scalar.activation(warm2[:], warm[:], EXP)

        cos_sb = const.tile([128, L], BF16)
        sin_sb = const.tile([128, L], BF16)
        nc.gpsimd.dma_start(cos_sb[:], cos_t[:])
        nc.gpsimd.dma_start(sin_sb[:], sin_t[:])

        qT = [qkv_pool.tile([128, L], BF16, tag=f"qT{b}", name=f"qT{b}")
              for b in range(B)]
        kT = [qkv_pool.tile([128, L], BF16, tag=f"kT{b}", name=f"kT{b}")
              for b in range(B)]
        vaug = [qkv_pool.tile([128, NJ * 130], BF16, tag=f"va{b}",
                              name=f"va{b}") for b in range(B)]

        with tc.tile_pool(name="xt", bufs=3) as xt_pool, \
             tc.tile_pool(name="rope", bufs=3) as rope_pool, \
             tc.tile_pool(name="pp", bufs=2, space="PSUM") as proj_psum, \
             tc.tile_pool(name="sp", bufs=2, space="PSUM") as s_psum, \
             tc.tile_pool(name="op", bufs=2, space="PSUM") as o_psum, \
             tc.tile_pool(name="pt", bufs=4) as p_pool, \
             tc.tile_pool(name="nrm", bufs=3) as n_pool:

            xT_d = xT.rearrange("(dmt p) s -> p dmt s", p=128)

            ot_r = [None, None]

            def load_ot(b):
                ot = p_pool.tile([128, 8 * 512], BF16, tag="ot", name="ot",
                                 bufs=2)
                r = ot.rearrange("p (ct s) -> p ct s", ct=8)
                nc.sync.dma_start(r[:],
                                  a2a_out[b][:].rearrange("ct p s -> p ct s"))
                ot_r[b] = r

            def outproj_block(b, st, nch):
                ps = proj_psum.tile([128, 512], F32, tag="ps", name="fps")
                for ct in range(8):
                    mm(ps[:], ot_r[b][:, ct, st * 128:(st + 1) * 128],
                       wout_r[:, ct, nch * 512:(nch + 1) * 512],
                       start=(ct == 0), stop=False)
                mm(ps[:], ones_row[:], bout[:, nch * 512:(nch + 1) * 512],
                   start=False, stop=True)
                ob = n_pool.tile([128, 512], F32, tag="ob", name="ob")
                nc.vector.tensor_copy(ob[:], ps[:])
                nc.sync.dma_start(
                    out[b * 512 + st * 128: b * 512 + (st + 1) * 128,
                        nch * 512:(nch + 1) * 512], ob[:])

            for b in range(B):
                _projection(nc, tc, b, xT_d, wqk_r, wv_r, bqk, bv, ones_row,
                            cos_sb, sin_sb, qT[b], kT[b], vaug[b],
                            xt_pool, rope_pool, proj_psum)
                if b == 0:
                    # 4MB cast-load sits after xt(b0) on the gpsimd queue,
                    # done long before the out-projection needs it
                    nc.gpsimd.dma_start(
                        wout_r[:],
                        w_out.rearrange("(ct p) n -> p ct n", p=128))
                _attention(nc, tc, b, qT[b], kT[b], vaug[b],
                           s_psum, o_psum, p_pool, n_pool, a2a_in,
                           dbg if b == 0 else None)
                nc.gpsimd.collective_compute(
                    "AllToAll", mybir.AluOpType.bypass,
                    replica_groups=[list(range(N_CORES))],
                    ins=[a2a_in[b].opt()], outs=[a2a_out[b].opt()])
                if dbg is not None and b == 0:
                    _dump_bf16(nc, tc, dbg["qt0"], qT[0])
                    _dump_bf16(nc, tc, dbg["kt0"], kT[0])
                    _dump_bf16(nc, tc, dbg["va0"], vaug[0])

            # out-proj b0 executes during the A2A-2 wait (deps ready since
            # A2A-1); psum comes from the proj 'ps' tag, long free
            load_ot(0)
            for blk in range(8):
                outproj_block(0, blk // 2, blk % 2)
            load_ot(1)
            load_ot(1)
            for blk in range(8):
                outproj_block(1, blk // 2, blk % 2)


def _projection(nc, tc, b, xT_d, wqk_r, wv_r, bqk, bv, ones_row,
                cos_sb, sin_sb, qTb, kTb, vaugb,
                xt_pool, rope_pool, proj_psum, qk_sel=(0, 1), only_xc=None):
    mm = nc.tensor.matmul
    do_v = 1 in qk_sel
    va4 = vaugb.rearrange("p (st h c) -> p st h c", st=NJ, h=2)
    if do_v:
        nc.vector.memset(va4[:, :, :, 64:65], 1.0)

    xcs = range(NXC) if only_xc is None else only_xc
    for xc in xcs:
        s0 = xc * XC
        xt = xt_pool.tile([128, 8, XC], BF16)
        nc.gpsimd.dma_start(xt[:], xT_d[:, :, b * L + s0: b * L + s0 + XC])

        for ct in qk_sel:
            ps = proj_psum.tile([128, XC], F32, tag="ps", name="ps")
            for dmt in range(8):
                mm(ps[:], wqk_r[:, ct, dmt, :], xt[:, dmt, :],
                   start=(dmt == 0), stop=(dmt == 7))
            tgt = qTb if ct == 0 else kTb
            qb = rope_pool.tile([128, XC], BF16, tag="qb")
            nc.vector.tensor_scalar_add(qb[:], ps[:], bqk[:, ct:ct + 1])
            rot = rope_pool.tile([128, XC], BF16, tag="rot")
            for h2 in range(4):
                src = h2 * 32 + (32 if h2 % 2 == 0 else -32)
                nc.sync.dma_start(rot[h2 * 32:(h2 + 1) * 32, :],
                                  qb[src:src + 32, :])
            tsin = rope_pool.tile([128, XC], BF16, tag="tsin")
            nc.vector.tensor_mul(tsin[:], rot[:], sin_sb[:, s0:s0 + XC])
            tcos = rope_pool.tile([128, XC], BF16, tag="tcos")
            nc.vector.tensor_mul(tcos[:], qb[:], cos_sb[:, s0:s0 + XC])
            nc.vector.tensor_add(tgt[:, s0:s0 + XC], tcos[:], tsin[:])

        for u in (range(XC // 128) if do_v else ()):
            st = (s0 + u * 128) // 128
            ps = proj_psum.tile([128, 128], F32, tag="ps", name="vps")
            for dmt in range(8):
                mm(ps[:], xt[:, dmt, u * 128:(u + 1) * 128],
                   wv_r[:, dmt, :], start=(dmt == 0), stop=False)
            mm(ps[:], ones_row[:], bv[:], start=False, stop=True)
            nc.vector.tensor_copy(
                va4[:, st, :, 0:64],
                ps[:].rearrange("p (h c) -> p h c", h=2))


def _dump_bf16(nc, tc, dst_dram, src_sb):
    from concourse.bass import MemorySpace
    if src_sb.space == MemorySpace.DRAM:
        nc.gpsimd.dma_start(dst_dram[:], src_sb[:])  # cast bf16 -> f32
        return
    nc.sync.dma_start(dst_dram[:], src_sb[:])  # bf16 -> bf16 direct


def _attention(nc, tc, b, qTb, kTb, vaugb, s_psum, o_psum, p_pool, n_pool,
               a2a_in, dbg=None, post_ic=None):
    mm = nc.tensor.matmul
    va = vaugb

    def mm1(j, ic, s_ps):
        for h in range(2):
            p0 = 64 * h
            mm(s_ps[:, h * 512:(h + 1) * 512],
               kTb[p0:p0 + 64, j * 128:(j + 1) * 128],
               qTb[p0:p0 + 64, ic * 512:(ic + 1) * 512],
               start=True, stop=True, tile_position=(p0, 0))

    for ic in range(NIC):
        o_ps = [o_psum.tile([65, 512], F32, tag="o", name=f"o{h}")
                for h in range(2)]
        s_tiles = {0: s_psum.tile([128, 1024], F32, name="s_t")}
        mm1(0, ic, s_tiles[0])
        for j in range(NJ):
            if j + 1 < NJ:
                s_tiles[j + 1] = s_psum.tile([128, 1024], F32, name="s_t")
                mm1(j + 1, ic, s_tiles[j + 1])
            p_sb = p_pool.tile([128, 1024], BF16)
            sc = 1024 - DVE_COLS
            nc.scalar.activation(p_sb[:, 0:sc], s_tiles[j][:, 0:sc],
                                 EXP, scale=0.125)
            if DVE_COLS:
                nc.vector.tensor_scalar(
                    p_sb[:, sc:1024].bitcast(I16), s_tiles[j][:, sc:1024],
                    SCH_C1, SCH_C2,
                    mybir.AluOpType.mult, mybir.AluOpType.add)
            if dbg is not None and ic == 0 and j == 0:
                dt0 = p_pool.tile([128, 1024], F32, tag="dbgp", name="dbgp")
                nc.vector.tensor_copy(dt0[:], p_sb[:])
                nc.sync.dma_start(dbg["p00"][:], dt0[:])
            for h in range(2):
                mm(o_ps[h][:],
                   va[:, j * 130 + h * 65: j * 130 + (h + 1) * 65],
                   p_sb[:, h * 512:(h + 1) * 512],
                   start=(j == 0), stop=(j == NJ - 1))
            del s_tiles[j]

        if dbg is not None and ic == 0:
            dto = n_pool.tile([65, 512], F32, tag="dbgo", name="dbgo")
            nc.vector.tensor_copy(dto[:], o_ps[0][:])
            nc.sync.dma_start(dbg["o00"][:], dto[:])
        for h in range(2):
            # single fast drain frees the PSUM bank; normalize off-PSUM
            od = n_pool.tile([128, 512], F32, tag="od")
            nc.vector.tensor_copy(od[0:65, :], o_ps[h][0:65, :])
            rs0 = n_pool.tile([1, 512], F32, tag="rs0")
            nc.sync.dma_start(rs0[:], od[64:65, :])
            recip0 = n_pool.tile([1, 512], F32, tag="recip0")
            nc.vector.reciprocal_approx_fast(recip0[:], rs0[:])
            bc = n_pool.tile([64, 512], F32, tag="bc")
            nc.gpsimd.partition_broadcast(bc[:], recip0[:])
            oN = n_pool.tile([64, 512], BF16, tag="oN")
            nc.vector.tensor_mul(oN[:], od[0:64, :], bc[:])
            nc.sync.dma_start(
                a2a_in[b][ic, 64 * h:64 * (h + 1), :], oN[:])
        if post_ic is not None:
            post_ic(ic)


def make_tables():
    f = np.arange(HALF, dtype=np.float64)
    freqs = THETA ** (-f / HALF)
    ang = SCALE * np.outer(np.arange(L, dtype=np.float64), freqs)
    c32 = np.cos(ang.T).astype(np.float32)
    s32 = np.sin(ang.T).astype(np.float32)
    cos128 = np.concatenate([c32, c32, c32, c32], axis=0)
    sin128 = np.concatenate([-s32, s32, -s32, s32], axis=0)
    return np.ascontiguousarray(cos128), np.ascontiguousarray(sin128)


def make_in_maps(x, w_qkv, b_qkv, w_out, b_out):
    x = np.asarray(x, dtype=np.float32)
    w_qkv = np.asarray(w_qkv, dtype=np.float32)
    b_qkv = np.asarray(b_qkv, dtype=np.float32)
    w_out = np.ascontiguousarray(np.asarray(w_out, dtype=np.float32))
    b_out = np.ascontiguousarray(np.asarray(b_out, dtype=np.float32))
    xT = np.ascontiguousarray(x.transpose(2, 0, 1).reshape(DM, B * L))
    cos128, sin128 = make_tables()
    in_maps = []
    for i in range(N_CORES):
        h0, h1 = 2 * i, 2 * i + 1

        def wslice(base):
            return [w_qkv[:, base + 64 * h0: base + 64 * h0 + 64],
                    w_qkv[:, base + 64 * h1: base + 64 * h1 + 64]]

        def bslice(base):
            return [b_qkv[base + 64 * h0: base + 64 * h0 + 64],
                    b_qkv[base + 64 * h1: base + 64 * h1 + 64]]

        w_qk = np.ascontiguousarray(
            np.concatenate(wslice(0) + wslice(DM), axis=1), dtype=np.float32)
        b_qk = np.ascontiguousarray(
            np.concatenate(bslice(0) + bslice(DM)), dtype=np.float32)
        w_v = np.ascontiguousarray(
            np.concatenate(wslice(2 * DM), axis=1), dtype=np.float32)
        b_v = np.ascontiguousarray(
            np.concatenate(bslice(2 * DM)), dtype=np.float32)
        in_maps.append({
            "xT": xT, "w_qk": w_qk, "b_qk": b_qk, "w_v": w_v, "b_v": b_v,
            "w_out": w_out, "b_out": b_out,
            "cos_t": cos128, "sin_t": sin128,
        })
    return in_maps


def gather_out(results):
    per_b = []
    for b in range(B):
        per_b.append(np.concatenate(
            [results[i]["out"][512 * b:512 * (b + 1)] for i in range(N_CORES)],
            axis=0))
    return np.stack(per_b, axis=0).astype(np.float32)


# ---------------- harness entry ----------------

_NC_CACHE = {}


def _run(x, w_qkv, b_qkv, w_out, b_out, trace=False):
    from concourse.bass_utils import run_bass_kernel_spmd

    if "nc" not in _NC_CACHE:
        _NC_CACHE["nc"] = build()
    nc = _NC_CACHE["nc"]
    in_maps = make_in_maps(x, w_qkv, b_qkv, w_out, b_out)
    res = run_bass_kernel_spmd(nc, in_maps, list(range(N_CORES)), trace=trace)
    return gather_out(res.results), res


def kernel(x, w_qkv, b_qkv, w_out, b_out):
    full, _ = _run(x, w_qkv, b_qkv, w_out, b_out, trace=False)
    return full

